# revision 1
# baseline (speedup 1.0000x reference)
"""Multi-head self-attention (RoPE + causal flash-style softmax) on 8 Trainium2
NeuronCores.

Sharding: head-parallel (Megatron). Core c owns heads {2c, 2c+1}:
  - Wq/Wk/Wv column-split -> each core projects its 128 features for all
    B*S = 4096 tokens (computed in transposed layout [feat, t] so the
    contraction dim of every matmul sits on SBUF partitions).
  - RoPE applied via a partition-swap permutation matmul + DVE elementwise.
  - Attention per (batch, head) in scores-transposed layout [kp, q]; softmax
    denominator comes from a ones-column appended to V in the same PSUM
    accumulation group; no max-subtraction (scores ~ N(0,1) at this scale).
  - Wo row-split -> per-core partial y [4096, 1024]; host sums the 8 partials.

All matmuls run in float32r (~1.5e-4 rel err, 4x fp32 throughput).
"""

import numpy as np

B = 2
S = 2048
D = 1024
H = 16
HD = 64
T = B * S  # 4096
P = 128
N_CORES = 8
KT = D // P  # 8 k-tiles for the projections
N_CH = T // 512  # 8 projection chunks of 512 tokens
QC_W = 512  # attention q-chunk width
N_QC = S // QC_W  # 8 q-chunks per (batch, head)
NEG = -1.0e9

_CACHE = {}


def _build():
    import concourse.bass as bass
    import concourse.mybir as mybir
    from concourse import bacc
    from concourse.bass import ts
    from concourse.tile import TileContext

    F32 = mybir.dt.float32
    F32R = mybir.dt.float32r
    EXP = mybir.ActivationFunctionType.Exp

    nc = bacc.Bacc("TRN2", target_bir_lowering=False, debug=False,
                   num_devices=N_CORES)

    xt = nc.dram_tensor("xt", [D, T], F32, kind="ExternalInput")
    wq = nc.dram_tensor("wq", [D, P], F32, kind="ExternalInput")
    wk = nc.dram_tensor("wk", [D, P], F32, kind="ExternalInput")
    wv = nc.dram_tensor("wv", [D, P], F32, kind="ExternalInput")
    wo = nc.dram_tensor("wo", [P, D], F32, kind="ExternalInput")
    cos = nc.dram_tensor("cos", [P, S], F32, kind="ExternalInput")
    sin = nc.dram_tensor("sin", [P, S], F32, kind="ExternalInput")
    perm = nc.dram_tensor("perm", [P, P], F32, kind="ExternalInput")
    ident = nc.dram_tensor("ident", [P, P], F32, kind="ExternalInput")
    cmask = nc.dram_tensor("cmask", [P, 896], F32, kind="ExternalInput")
    ones = nc.dram_tensor("ones", [P, 64], F32, kind="ExternalInput")
    y = nc.dram_tensor("y", [T, D], F32, kind="ExternalOutput")

    xt_r = xt[:, :].rearrange("(ko ki) t -> ki ko t", ki=P).bitcast(F32R)

    with TileContext(nc) as tc:
        with (
            tc.tile_pool(name="consts", bufs=1) as consts,
            tc.tile_pool(name="xtp", bufs=2) as xtp,
            tc.tile_pool(name="work", bufs=2) as work,
            tc.tile_pool(name="expp", bufs=8) as expp,
            tc.tile_pool(name="outp", bufs=3) as outp,
        ):
            # ---- resident tiles ----
            wq_sb = consts.tile([P, KT, P], F32R, tag="wq")
            wk_sb = consts.tile([P, KT, P], F32R, tag="wk")
            wv_sb = consts.tile([P, KT, P], F32R, tag="wv")
            wo_sb = consts.tile([P, D], F32R, tag="wo")
            cos_sb = consts.tile([P, S], F32, tag="cos")
            sin_sb = consts.tile([P, S], F32, tag="sin")
            perm_sb = consts.tile([P, P], F32R, tag="perm")
            id_sb = consts.tile([P, P], F32R, tag="ident")
            cm_sb = consts.tile([P, 896], F32, tag="cmask")
            rotq = consts.tile([P, T], F32R, tag="rotq")
            rotk = consts.tile([P, T], F32R, tag="rotk")
            # V in natural [kp, d] layout: [kp_part, kp_tile, head, 64 feat + 1 one]
            vall = consts.tile([P, T // P, 2, HD + 1], F32R, tag="vall")
            ones_row = consts.tile([1, HD], F32R, tag="ones_row")
            aot = consts.tile([P, T], F32R, tag="aot")  # attn out (transposed)

            nc.sync.dma_start(wq_sb[:], wq[:, :].rearrange(
                "(ko ki) f -> ki ko f", ki=P).bitcast(F32R))
            nc.sync.dma_start(wk_sb[:], wk[:, :].rearrange(
                "(ko ki) f -> ki ko f", ki=P).bitcast(F32R))
            nc.sync.dma_start(wv_sb[:], wv[:, :].rearrange(
                "(ko ki) f -> ki ko f", ki=P).bitcast(F32R))
            nc.sync.dma_start(wo_sb[:], wo[:, :].bitcast(F32R))
            nc.sync.dma_start(cos_sb[:], cos[:, :])
            nc.sync.dma_start(sin_sb[:], sin[:, :])
            nc.sync.dma_start(perm_sb[:], perm[:, :].bitcast(F32R))
            nc.sync.dma_start(id_sb[:], ident[:, :].bitcast(F32R))
            nc.sync.dma_start(cm_sb[:], cmask[:, :])
            nc.sync.dma_start(ones_row[:], ones[0:1, 0:HD].bitcast(F32R))
            nc.sync.dma_start(
                vall[:, :, :, HD],
                ones[:, :].rearrange("p (t h) -> p t h", t=T // P).bitcast(F32R))

            # ---- stage 1: projections + rope + V transpose ----
            stage1 = tc.tile_pool(name="pproj", bufs=1, space="PSUM")
            pproj = stage1.__enter__()
            stage1b = tc.tile_pool(name="pswp", bufs=1, space="PSUM")
            pswp = stage1b.__enter__()
            stage1c = tc.tile_pool(name="ptrp", bufs=1, space="PSUM")
            ptrp = stage1c.__enter__()
            for ch in range(N_CH):
                xt_t = xtp.tile([P, KT, 512], F32R, tag="xt")
                nc.sync.dma_start(xt_t[:, 0:KT // 2, :],
                                  xt_r[:, 0:KT // 2, ts(ch, 512)])
                nc.sync.dma_start(xt_t[:, KT // 2:KT, :],
                                  xt_r[:, KT // 2:KT, ts(ch, 512)])

                pq = pproj.tile([P, 512], F32, tag="pq")
                pk = pproj.tile([P, 512], F32, tag="pk")
                pv = pproj.tile([P, 512], F32, tag="pv")
                for k in range(KT):
                    st, sp = (k == 0), (k == KT - 1)
                    nc.tensor.matmul(pq[:], wq_sb[:, k, :], xt_t[:, k, :],
                                     start=st, stop=sp)
                for k in range(KT):
                    st, sp = (k == 0), (k == KT - 1)
                    nc.tensor.matmul(pk[:], wk_sb[:, k, :], xt_t[:, k, :],
                                     start=st, stop=sp)
                for k in range(KT):
                    st, sp = (k == 0), (k == KT - 1)
                    nc.tensor.matmul(pv[:], wv_sb[:, k, :], xt_t[:, k, :],
                                     start=st, stop=sp)

                qc_t = work.tile([P, 512], F32R, tag="qchunk")
                kc_t = work.tile([P, 512], F32R, tag="kchunk")
                vc_t = work.tile([P, 512], F32R, tag="vchunk")
                nc.scalar.copy(qc_t[:], pq[:])
                nc.scalar.copy(kc_t[:], pk[:])
                nc.scalar.copy(vc_t[:], pv[:])

                # V: transpose [feat, t] -> natural [t, feat] via PE
                for sub in range(4):
                    ptr_t = ptrp.tile([P, P], F32R, tag="ptr")
                    nc.tensor.transpose(ptr_t[:], vc_t[:, ts(sub, P)], id_sb[:])
                    nc.scalar.copy(
                        vall[:, ch * 4 + sub, :, 0:HD],
                        ptr_t[:].rearrange("p (h d) -> p h d", h=2))

                # RoPE: rot = q * cos + swap(q) * sin_signed
                s_sl = ts(ch % (S // 512), 512)
                for src, dst in ((qc_t, rotq), (kc_t, rotk)):
                    psw = pswp.tile([P, 512], F32, tag="psw")
                    nc.tensor.matmul(psw[:], perm_sb[:], src[:],
                                     start=True, stop=True)
                    t1 = work.tile([P, 512], F32, tag="ropet1")
                    t2 = work.tile([P, 512], F32, tag="ropet2")
                    nc.vector.tensor_mul(t1[:], src[:].bitcast(F32),
                                         cos_sb[:, s_sl])
                    nc.vector.tensor_mul(t2[:], psw[:], sin_sb[:, s_sl])
                    nc.vector.tensor_add(t1[:], t1[:], t2[:])
                    nc.scalar.copy(dst[:, ts(ch, 512)], t1[:])

            stage1c.__exit__(None, None, None)
            stage1b.__exit__(None, None, None)
            stage1.__exit__(None, None, None)

            # ---- stage 2: attention per (batch, local head) ----
            stage2 = tc.tile_pool(name="pss", bufs=4, space="PSUM")
            pssp = stage2.__enter__()
            stage2b = tc.tile_pool(name="pso", bufs=2, space="PSUM")
            psop = stage2b.__enter__()
            stage2c = tc.tile_pool(name="pbc", bufs=1, space="PSUM")
            pbcp = stage2c.__enter__()
            stage3 = tc.tile_pool(name="pyp", bufs=1, space="PSUM")
            pyp = stage3.__enter__()

            def wo_block(b):
                # output projection for batch b (aot[:, b*S:(b+1)*S] complete)
                for mi in range(S // P):
                    m = b * (S // P) + mi
                    for oc in range(2):
                        py = pyp.tile([P, 512], F32, tag="py")
                        nc.tensor.matmul(py[:], aot[:, ts(m, P)],
                                         wo_sb[:, ts(oc, 512)],
                                         start=True, stop=True)
                        y_sb = outp.tile([P, 512], F32, tag="ysb")
                        if oc == 0:
                            nc.vector.tensor_copy(y_sb[:], py[:])
                        else:
                            nc.scalar.copy(y_sb[:], py[:])
                        nc.sync.dma_start(y[ts(m, P), ts(oc, 512)], y_sb[:])

            for b in range(B):
                for hl in range(2):
                    pr = slice(HD * hl, HD * hl + HD)
                    t0 = b * S
                    for qc in range(N_QC):
                        ntk = 4 * (qc + 1)
                        ps_o = psop.tile([P, QC_W], F32, tag="pso")
                        q_sl = slice(t0 + QC_W * qc, t0 + QC_W * (qc + 1))
                        exps = []
                        for t in range(ntk):
                            ps_s = pssp.tile([P, QC_W], F32, tag="pss")
                            nc.tensor.matmul(
                                ps_s[:],
                                rotk[pr, t0 + P * t: t0 + P * (t + 1)],
                                rotq[pr, q_sl],
                                start=True, stop=True)
                            if t >= 4 * qc:  # diagonal band: additive mask
                                td = t - 4 * qc
                                w = P * (td + 1)
                                off = 384 - P * td
                                nc.vector.tensor_add(
                                    ps_s[:, 0:w], ps_s[:, 0:w],
                                    cm_sb[:, off:off + w])
                            e = expp.tile([P, QC_W], F32R, tag="expT")
                            nc.scalar.activation(e[:], ps_s[:], EXP, scale=0.125)
                            exps.append(e)
                        for t, e in enumerate(exps):
                            nc.tensor.matmul(
                                ps_o[0:HD + 1, :],
                                vall[:, b * (S // P) + t, hl, :],
                                e[:], start=(t == 0), stop=(t == ntk - 1),
                                skip_group_check=True)
                        dn = work.tile([1, QC_W], F32R, tag="denr")
                        nc.scalar.copy(dn[:], ps_o[HD:HD + 1, :])
                        pb = pbcp.tile([HD, QC_W], F32, tag="pbc")
                        nc.tensor.matmul(pb[:], ones_row[:], dn[:],
                                         start=True, stop=True)
                        rb_sb = work.tile([HD, QC_W], F32, tag="rbsb")
                        nc.vector.reciprocal_approx_fast(rb_sb[:], pb[:])
                        ao = work.tile([HD, QC_W], F32, tag="aof32")
                        nc.vector.tensor_mul(ao[:], ps_o[0:HD, :], rb_sb[:])
                        nc.scalar.copy(aot[pr, q_sl], ao[:])
                wo_block(b)

            stage3.__exit__(None, None, None)
            stage2c.__exit__(None, None, None)
            stage2b.__exit__(None, None, None)
            stage2.__exit__(None, None, None)

    nc.compile()
    return nc


def _host_prep(x, token_positions, Wq, Wk, Wv, Wo, rope_sin, rope_cos):
    x = np.asarray(x, dtype=np.float32)
    Wq = np.asarray(Wq, dtype=np.float32)
    Wk = np.asarray(Wk, dtype=np.float32)
    Wv = np.asarray(Wv, dtype=np.float32)
    Wo = np.asarray(Wo, dtype=np.float32)
    pos = np.asarray(token_positions).astype(np.int64)
    sin_g = np.asarray(rope_sin, dtype=np.float32)[pos]  # [S, 32]
    cos_g = np.asarray(rope_cos, dtype=np.float32)[pos]

    xt = np.ascontiguousarray(x.reshape(T, D).T)  # [D, T]

    j = np.arange(P) % 32
    cosE = np.ascontiguousarray(cos_g.T[j, :])  # [128, S]
    sgn = np.where((np.arange(P) % HD) < 32, -1.0, 1.0).astype(np.float32)
    sinS = np.ascontiguousarray(sgn[:, None] * sin_g.T[j, :])

    p_idx = np.arange(P)
    swap = (p_idx // HD) * HD + ((p_idx % HD) + 32) % HD
    perm = np.zeros((P, P), dtype=np.float32)
    perm[swap, p_idx] = 1.0
    ident = np.eye(P, dtype=np.float32)

    u = np.arange(896)[None, :]
    cmask = np.where(u >= (np.arange(P)[:, None] + 384), 0.0, NEG).astype(
        np.float32)
    ones = np.ones((P, 64), dtype=np.float32)

    in_maps = []
    for c in range(N_CORES):
        feats = []
        for hl in range(2):
            h = 2 * c + hl
            base = h * HD
            feats.extend(base + 2 * np.arange(32))      # x1 (even d)
            feats.extend(base + 2 * np.arange(32) + 1)  # x2 (odd d)
        feats = np.array(feats)
        nat = np.arange(2 * c * HD, (2 * c + 2) * HD)
        in_maps.append({
            "xt": xt,
            "wq": np.ascontiguousarray(Wq[feats, :].T),
            "wk": np.ascontiguousarray(Wk[feats, :].T),
            "wv": np.ascontiguousarray(Wv[nat, :].T),
            "wo": np.ascontiguousarray(Wo[:, nat].T),
            "cos": cosE, "sin": sinS, "perm": perm, "ident": ident,
            "cmask": cmask, "ones": ones,
        })
    return in_maps


def run(trace=False, **inputs):
    from concourse.bass_utils import run_bass_kernel_spmd

    if "nc" not in _CACHE:
        _CACHE["nc"] = _build()
    nc = _CACHE["nc"]
    in_maps = _host_prep(**inputs)
    res = run_bass_kernel_spmd(nc, in_maps, core_ids=list(range(N_CORES)),
                               trace=trace)
    out = np.zeros((T, D), dtype=np.float32)
    for c in range(N_CORES):
        out += res.results[c]["y"]
    return out.reshape(B, S, D), res


def kernel(**inputs) -> np.ndarray:
    out, _ = run(trace=False, **inputs)
    return out



# revision 2
# speedup vs baseline: 1.2898x; 1.2898x over previous
"""Multi-head self-attention (RoPE + causal softmax) on 8 Trainium2 NeuronCores.

Sharding: head-parallel (Megatron). Core c owns heads {2c, 2c+1}:
  - Wq/Wk/Wv column-split -> each core projects its 128 features for all
    B*S = 4096 tokens in transposed layout [feat, t] (contraction on SBUF
    partitions). All matmuls run in bf16 (1 cycle/row on the PE vs 2 for
    fp32r) with fp32 PSUM accumulation.
  - RoPE via a partition-swap permutation matmul + DVE elementwise.
  - Attention per (batch, head) in scores-transposed layout [kp, q].
    Causal-band score tiles are column-trimmed to the live q-range and
    packed into shared PSUM tiles so one exp instruction covers several
    k-tiles. Softmax denominator comes from a ones-column appended to V
    in the same PSUM accumulation group; no max-subtraction (scores are
    O(1) at this scale).
  - Wo row-split -> per-core partial y in fp16; host sums the 8 partials.
"""

import numpy as np

B = 2
S = 2048
D = 1024
H = 16
HD = 64
T = B * S  # 4096
P = 128
N_CORES = 8
KT = D // P  # 8 k-tiles for the projections
N_CH = T // 512  # 8 projection chunks of 512 tokens
QC_W = 512  # attention q-chunk width
N_QC = S // QC_W  # 4 q-chunks per (batch, head)
NEG = -1.0e9

_CACHE = {}


def _build():
    import concourse.bass as bass
    import concourse.mybir as mybir
    from concourse import bacc
    from concourse.bass import ts
    from concourse.tile import TileContext

    F32 = mybir.dt.float32
    F16 = mybir.dt.float16
    BF16 = mybir.dt.bfloat16
    EXP = mybir.ActivationFunctionType.Exp

    nc = bacc.Bacc("TRN2", target_bir_lowering=False, debug=False,
                   num_devices=N_CORES)

    xt = nc.dram_tensor("xt", [D, T], BF16, kind="ExternalInput")
    wq = nc.dram_tensor("wq", [D, P], BF16, kind="ExternalInput")
    wk = nc.dram_tensor("wk", [D, P], BF16, kind="ExternalInput")
    wv = nc.dram_tensor("wv", [D, P], BF16, kind="ExternalInput")
    wo = nc.dram_tensor("wo", [P, D], BF16, kind="ExternalInput")
    cos = nc.dram_tensor("cos", [P, S], F32, kind="ExternalInput")
    sin = nc.dram_tensor("sin", [P, S], F32, kind="ExternalInput")
    perm = nc.dram_tensor("perm", [P, P], BF16, kind="ExternalInput")
    ident = nc.dram_tensor("ident", [P, P], BF16, kind="ExternalInput")
    cmask = nc.dram_tensor("cmask", [P, P], F32, kind="ExternalInput")
    ones = nc.dram_tensor("ones", [P, 2 * (T // P)], BF16,
                          kind="ExternalInput")
    y = nc.dram_tensor("y", [T, D], F16, kind="ExternalOutput")

    xt_r = xt[:, :].rearrange("(ko ki) t -> ki ko t", ki=P)

    with TileContext(nc) as tc:
        with (
            tc.tile_pool(name="consts", bufs=1) as consts,
            tc.tile_pool(name="xtp", bufs=2) as xtp,
            tc.tile_pool(name="work", bufs=2) as work,
            tc.tile_pool(name="expp", bufs=10) as expp,
            tc.tile_pool(name="outp", bufs=3) as outp,
        ):
            # ---- resident tiles ----
            wq_sb = consts.tile([P, KT, P], BF16, tag="wq")
            wk_sb = consts.tile([P, KT, P], BF16, tag="wk")
            wv_sb = consts.tile([P, KT, P], BF16, tag="wv")
            wo_sb = consts.tile([P, D], BF16, tag="wo")
            cos_sb = consts.tile([P, S], F32, tag="cos")
            sin_sb = consts.tile([P, S], F32, tag="sin")
            perm_sb = consts.tile([P, P], BF16, tag="perm")
            id_sb = consts.tile([P, P], BF16, tag="ident")
            cm_sb = consts.tile([P, P], F32, tag="cmask")
            rotq = consts.tile([P, T], BF16, tag="rotq")
            rotk = consts.tile([P, T], BF16, tag="rotk")
            # V in natural [kp, d] layout: [kp_part, kp_tile, head, 64 + 1 one]
            vall = consts.tile([P, T // P, 2, HD + 1], BF16, tag="vall")
            ones_row = consts.tile([1, HD], BF16, tag="ones_row")
            aot = consts.tile([P, T], BF16, tag="aot")  # attn out (transposed)

            nc.sync.dma_start(wq_sb[:], wq[:, :].rearrange(
                "(ko ki) f -> ki ko f", ki=P))
            nc.sync.dma_start(wk_sb[:], wk[:, :].rearrange(
                "(ko ki) f -> ki ko f", ki=P))
            nc.sync.dma_start(wv_sb[:], wv[:, :].rearrange(
                "(ko ki) f -> ki ko f", ki=P))
            nc.sync.dma_start(wo_sb[:], wo[:, :])
            nc.sync.dma_start(cos_sb[:], cos[:, :])
            nc.sync.dma_start(sin_sb[:], sin[:, :])
            nc.sync.dma_start(perm_sb[:], perm[:, :])
            nc.sync.dma_start(id_sb[:], ident[:, :])
            nc.sync.dma_start(cm_sb[:], cmask[:, :])
            nc.sync.dma_start(ones_row[:], ones[0:1, 0:HD])
            nc.sync.dma_start(
                vall[:, :, :, HD],
                ones[:, :].rearrange("p (t h) -> p t h", t=T // P))

            # ---- stage 1: projections + rope + V transpose ----
            stage1 = tc.tile_pool(name="pproj", bufs=1, space="PSUM")
            pproj = stage1.__enter__()
            stage1b = tc.tile_pool(name="pswp", bufs=2, space="PSUM")
            pswp = stage1b.__enter__()
            stage1c = tc.tile_pool(name="ptrp", bufs=2, space="PSUM")
            ptrp = stage1c.__enter__()
            for ch in range(N_CH):
                xt_t = xtp.tile([P, KT, 512], BF16, tag="xt")
                nc.sync.dma_start(xt_t[:, 0:KT // 2, :],
                                  xt_r[:, 0:KT // 2, ts(ch, 512)])
                nc.sync.dma_start(xt_t[:, KT // 2:KT, :],
                                  xt_r[:, KT // 2:KT, ts(ch, 512)])

                # pv first: its consumer chain (ACT copy -> PE transpose)
                # overlaps the pq/pk matmuls.
                pv = pproj.tile([P, 512], F32, tag="pv")
                pq = pproj.tile([P, 512], F32, tag="pq")
                pk = pproj.tile([P, 512], F32, tag="pk")
                for k in range(KT):
                    st, sp = (k == 0), (k == KT - 1)
                    nc.tensor.matmul(pv[:], wv_sb[:, k, :], xt_t[:, k, :],
                                     start=st, stop=sp)
                vc_t = work.tile([P, 512], BF16, tag="vchunk")
                nc.scalar.copy(vc_t[:], pv[:])
                for k in range(KT):
                    st, sp = (k == 0), (k == KT - 1)
                    nc.tensor.matmul(pq[:], wq_sb[:, k, :], xt_t[:, k, :],
                                     start=st, stop=sp)
                qc_t = work.tile([P, 512], BF16, tag="qchunk")
                nc.scalar.copy(qc_t[:], pq[:])
                for k in range(KT):
                    st, sp = (k == 0), (k == KT - 1)
                    nc.tensor.matmul(pk[:], wk_sb[:, k, :], xt_t[:, k, :],
                                     start=st, stop=sp)
                kc_t = work.tile([P, 512], BF16, tag="kchunk")
                nc.scalar.copy(kc_t[:], pk[:])

                # V: transpose [feat, t] -> natural [t, feat] via PE
                for sub in range(4):
                    ptr_t = ptrp.tile([P, P], BF16, tag="ptr")
                    nc.tensor.transpose(ptr_t[:], vc_t[:, ts(sub, P)],
                                        id_sb[:])
                    nc.vector.tensor_copy(
                        vall[:, ch * 4 + sub, :, 0:HD],
                        ptr_t[:].rearrange("p (h d) -> p h d", h=2))

                # RoPE: rot = proj * cos + swap(proj) * sin_signed
                s_sl = ts(ch % (S // 512), 512)
                for src_ps, src_sb, dst in ((pq, qc_t, rotq),
                                            (pk, kc_t, rotk)):
                    psw = pswp.tile([P, 512], F32, tag="psw")
                    nc.tensor.matmul(psw[:], perm_sb[:], src_sb[:],
                                     start=True, stop=True)
                    t1 = work.tile([P, 512], F32, tag="ropet1")
                    t2 = work.tile([P, 512], F32, tag="ropet2")
                    nc.vector.tensor_mul(t1[:], src_ps[:], cos_sb[:, s_sl])
                    nc.vector.tensor_mul(t2[:], psw[:], sin_sb[:, s_sl])
                    nc.vector.tensor_add(dst[:, ts(ch, 512)], t1[:], t2[:])

            stage1c.__exit__(None, None, None)
            stage1b.__exit__(None, None, None)
            stage1.__exit__(None, None, None)

            # ---- stage 2: attention per (batch, local head) ----
            # PSUM: pss2 pairs (2 banks x 2 bufs) + mix pool (ps_o/pb/py).
            stage2 = tc.tile_pool(name="pss2", bufs=2, space="PSUM")
            pssp = stage2.__enter__()
            stage2b = tc.tile_pool(name="pso", bufs=2, space="PSUM")
            psop = stage2b.__enter__()
            stage2c = tc.tile_pool(name="pbc", bufs=1, space="PSUM")
            pbcp = stage2c.__enter__()
            stage3 = tc.tile_pool(name="pyp", bufs=1, space="PSUM")
            pyp = stage3.__enter__()

            def wo_block(b):
                # output projection for batch b (aot[:, b*S:(b+1)*S] complete)
                for mi in range(S // P):
                    m = b * (S // P) + mi
                    for oc in range(2):
                        py = pyp.tile([P, 512], F32, tag="py")
                        nc.tensor.matmul(py[:], aot[:, ts(m, P)],
                                         wo_sb[:, ts(oc, 512)],
                                         start=True, stop=True)
                        y_sb = outp.tile([P, 512], F16, tag="ysb")
                        if oc == 0:
                            nc.vector.tensor_copy(y_sb[:], py[:])
                        else:
                            nc.scalar.copy(y_sb[:], py[:])
                        nc.sync.dma_start(y[ts(m, P), ts(oc, 512)], y_sb[:])

            for b in range(B):
                for hl in range(2):
                    pr = slice(HD * hl, HD * hl + HD)
                    t0 = b * S
                    for qc in range(N_QC):
                        ntk = 4 * (qc + 1)
                        q0 = t0 + QC_W * qc
                        ps_o = psop.tile([P, QC_W], F32, tag="pso")
                        # segs: (e_tile, half, c0, c1, qoff, t)
                        segs = []
                        # full k-tiles, paired two per PSUM tile
                        for p2 in range(2 * qc):
                            ps2 = pssp.tile([P, 2, 512], F32, tag="pss")
                            e2 = expp.tile([P, 2, 512], BF16, tag="expT")
                            for h2 in range(2):
                                t = 2 * p2 + h2
                                nc.tensor.matmul(
                                    ps2[:, h2, :],
                                    rotk[pr, t0 + P * t: t0 + P * (t + 1)],
                                    rotq[pr, q0:q0 + 512],
                                    start=True, stop=True)
                                segs.append((e2, h2, 0, 512, 0, t))
                            nc.scalar.activation(e2[:], ps2[:], EXP,
                                                 scale=0.125)
                        # causal band: tiles 4qc..4qc+3 trimmed to live
                        # columns; td0 + (td1,td3) packed in tile A,
                        # td2 in tile B.
                        psA = pssp.tile([P, 2, 512], F32, tag="pss")
                        eA = expp.tile([P, 2, 512], BF16, tag="expT")
                        for td, h2, c0, c1, qoff in (
                                (0, 0, 0, 512, 0),
                                (1, 1, 0, 384, 128),
                                (3, 1, 384, 512, 384)):
                            t = 4 * qc + td
                            nc.tensor.matmul(
                                ps2 := psA[:, h2, c0:c1],
                                rotk[pr, t0 + P * t: t0 + P * (t + 1)],
                                rotq[pr, q0 + qoff:q0 + 512],
                                start=True, stop=True)
                            del ps2
                            # triangle mask on the 128 cols at the diagonal
                            mc = c0 if td != 1 else 0
                            nc.vector.tensor_add(
                                psA[:, h2, mc:mc + P], psA[:, h2, mc:mc + P],
                                cm_sb[:, :])
                            segs.append((eA, h2, c0, c1, qoff, t))
                        nc.scalar.activation(eA[:], psA[:], EXP, scale=0.125)

                        psB = pssp.tile([P, 2, 512], F32, tag="pss")
                        eB = expp.tile([P, 2, 512], BF16, tag="expT")
                        t = 4 * qc + 2
                        nc.tensor.matmul(
                            psB[:, 0, 0:256],
                            rotk[pr, t0 + P * t: t0 + P * (t + 1)],
                            rotq[pr, q0 + 256:q0 + 512],
                            start=True, stop=True)
                        nc.vector.tensor_add(psB[:, 0, 0:P],
                                             psB[:, 0, 0:P], cm_sb[:, :])
                        nc.scalar.activation(eB[:, 0, 0:256],
                                             psB[:, 0, 0:256], EXP,
                                             scale=0.125)
                        segs.append((eB, 0, 0, 256, 256, t))

                        segs.sort(key=lambda s: s[5])
                        nseg = len(segs)
                        for i, (e2, h2, c0, c1, qoff, t) in enumerate(segs):
                            w = c1 - c0
                            nc.tensor.matmul(
                                ps_o[0:HD + 1, qoff:qoff + w],
                                vall[:, b * (S // P) + t, hl, :],
                                e2[:, h2, c0:c1],
                                start=(i == 0), stop=(i == nseg - 1),
                                skip_group_check=True)

                        dn = work.tile([1, QC_W], BF16, tag="denr")
                        nc.vector.tensor_copy(dn[:], ps_o[HD:HD + 1, :])
                        pb = pbcp.tile([HD, QC_W], F32, tag="pbc")
                        nc.tensor.matmul(pb[:], ones_row[:], dn[:],
                                         start=True, stop=True)
                        rb_sb = work.tile([HD, QC_W], F32, tag="rbsb")
                        nc.vector.reciprocal_approx_fast(rb_sb[:], pb[:])
                        q_sl = slice(q0, q0 + QC_W)
                        nc.vector.tensor_mul(aot[pr, q_sl],
                                             ps_o[0:HD, :], rb_sb[:])
                wo_block(b)

            stage3.__exit__(None, None, None)
            stage2c.__exit__(None, None, None)
            stage2b.__exit__(None, None, None)
            stage2.__exit__(None, None, None)

    nc.compile()
    return nc


def _host_prep(x, token_positions, Wq, Wk, Wv, Wo, rope_sin, rope_cos):
    import ml_dtypes
    bf16 = ml_dtypes.bfloat16

    x = np.asarray(x, dtype=np.float32)
    Wq = np.asarray(Wq, dtype=np.float32)
    Wk = np.asarray(Wk, dtype=np.float32)
    Wv = np.asarray(Wv, dtype=np.float32)
    Wo = np.asarray(Wo, dtype=np.float32)
    pos = np.asarray(token_positions).astype(np.int64)
    sin_g = np.asarray(rope_sin, dtype=np.float32)[pos]  # [S, 32]
    cos_g = np.asarray(rope_cos, dtype=np.float32)[pos]

    xt = np.ascontiguousarray(x.reshape(T, D).T).astype(bf16)  # [D, T]

    j = np.arange(P) % 32
    cosE = np.ascontiguousarray(cos_g.T[j, :])  # [128, S]
    sgn = np.where((np.arange(P) % HD) < 32, -1.0, 1.0).astype(np.float32)
    sinS = np.ascontiguousarray(sgn[:, None] * sin_g.T[j, :])

    p_idx = np.arange(P)
    swap = (p_idx // HD) * HD + ((p_idx % HD) + 32) % HD
    perm = np.zeros((P, P), dtype=np.float32)
    perm[swap, p_idx] = 1.0
    ident = np.eye(P, dtype=np.float32)

    # triangle mask: cm[p, j] masks iff j < p (q-local j, k-local p)
    u = np.arange(P)[None, :]
    cmask = np.where(u >= np.arange(P)[:, None], 0.0, NEG).astype(np.float32)
    ones = np.ones((P, 2 * (T // P)), dtype=np.float32)

    in_maps = []
    for c in range(N_CORES):
        feats = []
        for hl in range(2):
            h = 2 * c + hl
            base = h * HD
            feats.extend(base + 2 * np.arange(32))      # x1 (even d)
            feats.extend(base + 2 * np.arange(32) + 1)  # x2 (odd d)
        feats = np.array(feats)
        nat = np.arange(2 * c * HD, (2 * c + 2) * HD)
        in_maps.append({
            "xt": xt,
            "wq": np.ascontiguousarray(Wq[feats, :].T).astype(bf16),
            "wk": np.ascontiguousarray(Wk[feats, :].T).astype(bf16),
            "wv": np.ascontiguousarray(Wv[nat, :].T).astype(bf16),
            "wo": np.ascontiguousarray(Wo[:, nat].T).astype(bf16),
            "cos": cosE, "sin": sinS,
            "perm": perm.astype(bf16), "ident": ident.astype(bf16),
            "cmask": cmask, "ones": ones.astype(bf16),
        })
    return in_maps


def run(trace=False, **inputs):
    from concourse.bass_utils import run_bass_kernel_spmd

    if "nc" not in _CACHE:
        _CACHE["nc"] = _build()
    nc = _CACHE["nc"]
    in_maps = _host_prep(**inputs)
    res = run_bass_kernel_spmd(nc, in_maps, core_ids=list(range(N_CORES)),
                               trace=trace)
    out = np.zeros((T, D), dtype=np.float32)
    for c in range(N_CORES):
        out += res.results[c]["y"].astype(np.float32)
    return out.reshape(B, S, D), res


def kernel(**inputs) -> np.ndarray:
    out, _ = run(trace=False, **inputs)
    return out
